# revision 19
# baseline (speedup 1.0000x reference)
"""Trainium2 Bass kernel for CentroidEdgeConvNet (2-layer mean-aggregation GNN).

Reference computation (N=100000 nodes, DEG=16, F=H=128, C=40):
    h1 = relu(mean_k feats[nbr[i,k]] @ W0 + b0)            # [N, H]
    out = log_softmax(mean_k h1[nbr2[i,k]] @ W1 + b1)      # [N, C],  nbr2 = neighbors[ids]

v4: gather-only design (no dma_scatter_add).  Per phase, each output node's
DEG=16 source rows are DMA-gathered into per-node SBUF slots and summed
on-chip (DVE strided tensor_reduce), feeding the per-tile matmul pipeline.
The 1/16 mean and b1 are folded into W0/W1/b1 on the host.

dma_gather indices are int16, so sources are bucketed into 4 overlapping
32768-row windows.  Window overlap (~10k rows) lets ~30% of edges choose
between two buckets; a host-side greedy balance plus snake-sort of nodes by
bucket-count profile makes per-tile per-bucket budgets near-uniform, so
zero-row padding costs only ~12% extra gather descriptors.  Budgets (and
hence the compiled program) are derived from the actual input inside
kernel(); the compile is cached per budget signature.

Nodes are assigned to (core, tile, partition) by sorted rank: rank r ->
core (r//128)%8, tile r//1024, partition r%128.  All 8 cores share one SPMD
program; per-tile budgets are the max over the tile's 1024 ranks.  Phase 1
writes z = relu(h1) @ (W1/16) + b1/16 (padded to 64 f32) to a per-core shard
[12576, 64] (last 32 rows zero, used as in-window zero rows for phase-2
padding); an AllGather builds the 100608-row z table; phase 2 gathers and
reduces the same way and finishes with log_softmax.
"""

import numpy as np

import concourse.bacc as bacc
import concourse.mybir as mybir
import concourse.tile as tile
from concourse.bass_utils import run_bass_kernel_spmd
from concourse.masks import make_identity

# Problem constants (hardcoded per harness contract)
N_NODES = 100000
DEG = 16
F = 128
H = 128
C = 40
NCORES = 8
P = 128

TILES = 98                    # global tiles of 1024 ranks (8 cores x 128)
NRANK = TILES * 1024          # 100352 rank slots (100000 real + pad)
ZPAD = 64                     # z rows padded to 64 f32 = 256B
SROWS = TILES * P             # 12544 real z rows per core
ZTAIL = 32                    # zero rows appended per shard
SH_ROWS = SROWS + ZTAIL       # 12576
ZT_ROWS = SH_ROWS * NCORES    # 100608 rows in the all-gathered z table

WIN = 32768                   # int16 gather window
NB = 4

# phase-1 source table: feats with a zero row inserted after each 25000 block
T1_BLOCK = 25000
T1_ROWS = 100004
BASES1 = (0, 22412, 44824, 67236)
ZROWS1 = (25000, 25000, 50001, 75002)   # a zero row inside each window

# phase-2 source table: the all-gathered z table (zero rows = shard tails)
BASES2 = (0, 22614, 45228, 67840)
ZROWS2 = (12544, 25120, 50272, 75424)

CAP_COLS1 = 112               # max gather-buffer cols per group (phase 1, 512B)
CAP_COLS2 = 224               # phase 2 (256B)
NTG1 = 8                      # max tiles per group
NTG2 = 16
QUEUES = 4
MAXI = 1024                   # max indices per dma_gather op: the SWDGE
                              # descriptor ring fits ~65-80 descs/queue; an op
                              # needs num_idxs/16+1 (1024 -> 65 ok, 1280 -> 81
                              # hangs the device).

F32 = mybir.dt.float32
I16 = mybir.dt.int16


# --------------------------------------------------------------------------
# Host-side planning: bucket balancing, node ordering, budgets, indices
# --------------------------------------------------------------------------

def _balance(pos, bases):
    """Assign each edge to an eligible int16 window, balancing per-node counts.

    pos: [n, DEG] source-table positions.  Returns (basgn [n, DEG],
    prof [n, NB]).
    """
    n = pos.shape[0]
    b_arr = np.asarray(bases)
    elig = (pos[..., None] >= b_arr) & (pos[..., None] < b_arr + WIN)
    nelig = elig.sum(-1)
    assert (nelig >= 1).all()
    forced = nelig == 1
    k_lo = np.argmax(elig, -1)
    prof = np.zeros((n, NB), np.int32)
    rows = np.repeat(np.arange(n), DEG)[forced.ravel()]
    np.add.at(prof, (rows, k_lo[forced].ravel()), 1)
    basgn = np.where(forced, k_lo, -1)
    flex = nelig == 2
    ar = np.arange(n)
    for j in range(DEG):
        f = flex[:, j]
        kl = k_lo[:, j]
        kh = np.minimum(kl + 1, NB - 1)
        lo = prof[ar, kl]
        hi = prof[ar, kh]
        pick = np.where(lo <= hi, kl, kh)
        sel = np.where(f)[0]
        basgn[sel, j] = pick[sel]
        np.add.at(prof, (sel, pick[sel]), 1)
    assert (basgn >= 0).all()
    return basgn, prof


def _snake_order(prof):
    """Order nodes so nearby ranks have similar bucket profiles."""
    key0 = prof[:, 0]
    key1 = np.where(key0 % 2 == 0, prof[:, 1], DEG - prof[:, 1])
    key2 = np.where(key1 % 2 == 0, prof[:, 2], DEG - prof[:, 2])
    return np.lexsort((key2, key1, key0))


def _budgets(prof, order):
    """Per-tile per-bucket budgets: max over the tile's 1024 ranks, >=1."""
    n = prof.shape[0]
    sp = np.zeros((NRANK, NB), np.int32)
    sp[:n] = prof[order]
    W = sp.reshape(TILES, 1024, NB).max(axis=1)
    return np.maximum(W, 1)


def _groups(W, cap_cols, ntg_cap):
    """Partition tiles into consecutive groups bounded by cols and count."""
    groups = []
    t0, cols = 0, 0
    for t in range(TILES):
        ct = int(W[t].sum())
        if t > t0 and (cols + ct > cap_cols or t - t0 >= ntg_cap):
            groups.append((t0, t))
            t0, cols = t, 0
        cols += ct
    groups.append((t0, TILES))
    return groups


def _edge_table(pos, basgn, prof, zrows):
    """Per-node bucket-sorted positions, padded with the window's zero row.

    Returns ebp [n+1, NB, DEG] int32; row n (the pad node) is all zero rows.
    """
    n = pos.shape[0]
    order_k = np.argsort(basgn, axis=1, kind="stable")
    pos_s = np.take_along_axis(pos, order_k, axis=1)
    b_s = np.take_along_axis(basgn, order_k, axis=1)
    cum = np.cumsum(prof, axis=1)
    start = cum - prof
    within = np.arange(DEG)[None, :] - np.take_along_axis(start, b_s, axis=1)
    ebp = np.empty((n + 1, NB, DEG), np.int32)
    for b in range(NB):
        ebp[:, b, :] = zrows[b]
    ebp[np.arange(n)[:, None], b_s, within] = pos_s
    return ebp


def _wrap16(flat):
    """int16 idx layout: value j at partition j%16, col j//16, replicated x8."""
    arr = flat.reshape(-1, 16).T.astype(np.int16)
    return np.ascontiguousarray(np.tile(arr, (8, 1)))


def _build_idx(order, ebp, W, groups, bases, core):
    """Gather-index array for one core and one phase: [128, totcols*8] int16."""
    n = len(order)
    tot = int(W.sum())
    out = np.empty((tot, P), np.int32)
    col = 0
    pcols = np.arange(P)
    for (t0, t1) in groups:
        for b in range(NB):
            for t in range(t0, t1):
                r = t * 1024 + core * P + pcols
                nodes = np.where(r < n, order[np.minimum(r, n - 1)], n)
                w = int(W[t, b])
                vals = ebp[nodes, b, :w] - bases[b]       # [128, w]
                out[col:col + w, :] = vals.T
                col += w
    assert col == tot
    assert out.min() >= 0 and out.max() < WIN
    return _wrap16(out.reshape(-1))


def _plan(pos, bases, zrows):
    basgn, prof = _balance(pos, bases)
    order = _snake_order(prof)
    W = _budgets(prof, order)
    ebp = _edge_table(pos, basgn, prof, zrows)
    return order, W, ebp


# --------------------------------------------------------------------------
# Program builder
# --------------------------------------------------------------------------

def build_program(W1, G1, W2, G2, phases="full", maxi=None, repeat=1):
    """Build the SPMD Bass program from per-tile bucket budgets + groups.

    phases: "full" | "p1" (phase 1 only, z copied to out for verification) |
            "p1g" (phase-1 gathers only, out memset).
    maxi: optional cap on indices per dma_gather op (ops split at col
          boundaries).
    repeat: run the selected pipeline N times (timing amplification).
    """
    tot1 = int(W1.sum())
    tot2 = int(W2.sum())

    nc = bacc.Bacc(
        "TRN2", target_bir_lowering=False, debug=False, num_devices=NCORES,
        num_swdge_queues=QUEUES,
    )

    feats_t = nc.dram_tensor("feats", [T1_ROWS, F], F32, kind="ExternalInput")
    w0_t = nc.dram_tensor("w0", [F, H], F32, kind="ExternalInput")
    b0_t = nc.dram_tensor("b0", [H, 1], F32, kind="ExternalInput")
    w1_t = nc.dram_tensor("w1", [H, C], F32, kind="ExternalInput")
    b1_t = nc.dram_tensor("b1", [C, 1], F32, kind="ExternalInput")
    i1_t = nc.dram_tensor("i1", [P, tot1 * 8], I16, kind="ExternalInput")
    i2_t = nc.dram_tensor("i2", [P, tot2 * 8], I16, kind="ExternalInput")
    out_t = nc.dram_tensor("out", [P, TILES * C], F32, kind="ExternalOutput")

    AF = mybir.ActivationFunctionType
    ALU = mybir.AluOpType
    AX = mybir.AxisListType

    with tile.TileContext(nc) as tc:
        with (
            tc.tile_pool(name="const", bufs=1) as cpool,
            tc.tile_pool(name="gath", bufs=2) as gpool,
            tc.tile_pool(name="idx", bufs=2) as ipool,
            tc.tile_pool(name="part", bufs=2) as prpool,
            tc.tile_pool(name="msum", bufs=2) as mpool,
            tc.tile_pool(name="work", bufs=3) as wpool,
            tc.tile_pool(name="small", bufs=2) as spool,
            tc.tile_pool(name="outp", bufs=1) as opool,
            tc.tile_pool(name="ps", bufs=2, space="PSUM") as pspool,
            tc.tile_pool(name="dram", bufs=1, space="DRAM") as dpool,
        ):
            # --- constants / parameters ---
            w0_sb = cpool.tile([F, H], F32, name="w0_sb")
            nc.sync.dma_start(w0_sb[:], w0_t.ap())
            w1_sb = cpool.tile([H, C], F32, name="w1_sb")
            nc.sync.dma_start(w1_sb[:], w1_t.ap())
            b0_sb = cpool.tile([H, 1], F32, name="b0_sb")
            nc.sync.dma_start(b0_sb[:], b0_t.ap())
            b1_sb = cpool.tile([C, 1], F32, name="b1_sb")
            nc.sync.dma_start(b1_sb[:], b1_t.ap())
            ident = cpool.tile([P, P], F32, name="ident")
            make_identity(nc, ident[:])
            zzero = cpool.tile([P, ZPAD], F32, name="zzero")
            nc.vector.memset(zzero[:], 0.0)

            out_acc = opool.tile([P, TILES * C], F32, name="out_acc")

            z_shard = dpool.tile([SH_ROWS, ZPAD], F32, name="z_shard")

            def gather_group(W, g, t0, t1, idx_t, colbase, src_ap, elem, label):
                """Load idx slice + 4 bucketed gathers into a group buffer."""
                ntg = t1 - t0
                gcols = int(W[t0:t1].sum())
                ig = ipool.tile([P, gcols * 8], I16, name=f"ig{label}", tag="ix")
                nc.sync.dma_start(
                    ig[:], idx_t.ap()[:, colbase * 8:(colbase + gcols) * 8]
                )
                GB = None
                if phases != "p1gsep":
                    GB = gpool.tile([P, gcols, elem], F32,
                                    name=f"gb{label}", tag="gb")
                off = 0
                boffs = []
                qn = 0
                for b in range(NB):
                    bc = int(W[t0:t1, b].sum())
                    boffs.append(off)
                    step = bc if maxi is None else max(1, maxi // P)
                    for o2 in range(0, bc, step):
                        pc = min(step, bc - o2)
                        ni = pc * P
                        if phases == "p1gsep":
                            gsep = gpool.tile(
                                [P, pc, elem], F32, name="gsep", tag="gsep",
                                bufs=32,
                            )
                            dst = gsep[:]
                        else:
                            dst = GB[:, off + o2:off + o2 + pc, :]
                        nc.gpsimd.dma_gather(
                            out_ap=dst,
                            in_ap=src_ap[b],
                            idxs_ap=ig[:, (off + o2) * 8:(off + o2 + pc) * 8],
                            num_idxs=ni,
                            num_idxs_reg=ni,
                            elem_size=elem,
                            queue_num=qn % QUEUES,
                        )
                        qn += 1
                    off += bc
                return GB, boffs, gcols, ntg

            def reduce_group(W, t0, t1, GB, boffs, elem, ntg, label):
                """Strided tensor_reduce per equal-budget run, then combine."""
                PT = prpool.tile([P, ntg, NB, elem], F32, name=f"pt{label}",
                                 tag="part")
                for b in range(NB):
                    o = boffs[b]
                    t = t0
                    while t < t1:
                        w = int(W[t, b])
                        te = t
                        while te < t1 and int(W[te, b]) == w:
                            te += 1
                        nrun = te - t
                        src = GB[:, o:o + nrun * w, :].rearrange(
                            "p (t w) f -> p t f w", w=w
                        )
                        nc.vector.tensor_reduce(
                            out=PT[:, t - t0:te - t0, b, :],
                            in_=src,
                            axis=AX.X,
                            op=ALU.add,
                        )
                        o += nrun * w
                        t = te
                S = mpool.tile([P, ntg, elem], F32, name=f"s{label}", tag="s")
                M = mpool.tile([P, ntg, elem], F32, name=f"m{label}", tag="m")
                nc.vector.tensor_tensor(
                    out=S[:], in0=PT[:, :, 0, :], in1=PT[:, :, 1, :],
                    op=ALU.add,
                )
                nc.vector.tensor_tensor(
                    out=M[:], in0=PT[:, :, 2, :], in1=PT[:, :, 3, :],
                    op=ALU.add,
                )
                nc.vector.tensor_tensor(
                    out=M[:], in0=M[:], in1=S[:], op=ALU.add,
                )
                return M

            # ---------------- phase 1 ----------------
            def phase1():
              src1 = [feats_t.ap()[BASES1[b]:, :] for b in range(NB)]
              colbase = 0
              for g, (t0, t1) in enumerate(G1):
                GB, boffs, gcols, ntg = gather_group(
                    W1, g, t0, t1, i1_t, colbase, src1, F, f"1_{g}"
                )
                if phases in ("p1g", "p1gsep"):
                    colbase += gcols
                    continue
                M = reduce_group(W1, t0, t1, GB, boffs, F, ntg, f"1_{g}")
                for t in range(t0, t1):
                    tl = t - t0
                    m1t_p = pspool.tile([P, P], F32, name="m1t_p", tag="mt_p")
                    nc.tensor.transpose(m1t_p[:], M[:, tl, :], ident[:])
                    m1t = wpool.tile([P, P], F32, name="m1t", tag="mt")
                    nc.scalar.copy(m1t[:], m1t_p[:])
                    h1t_p = pspool.tile([H, P], F32, name="h1t_p", tag="h1_p")
                    nc.tensor.matmul(
                        h1t_p[:], lhsT=w0_sb[:], rhs=m1t[:],
                        start=True, stop=True,
                    )
                    h1t = wpool.tile([H, P], F32, name="h1t", tag="h1")
                    nc.scalar.activation(
                        h1t[:], h1t_p[:], AF.Relu, bias=b0_sb[:, 0:1]
                    )
                    z_p = pspool.tile([C, P], F32, name="z_p", tag="z_p")
                    nc.tensor.matmul(
                        z_p[:], lhsT=w1_sb[:], rhs=h1t[:],
                        start=True, stop=True,
                    )
                    zc = wpool.tile([C, P], F32, name="zc", tag="zc")
                    nc.scalar.activation(
                        zc[:], z_p[:], AF.Identity, bias=b1_sb[:, 0:1]
                    )
                    zt_p = pspool.tile([P, C], F32, name="zt_p", tag="zt_p")
                    nc.tensor.transpose(zt_p[:], zc[:], ident[:C, :C])
                    zt = wpool.tile([P, ZPAD], F32, name="zt", tag="zt")
                    nc.gpsimd.memset(zt[:, C:], 0.0)
                    nc.scalar.copy(zt[:, :C], zt_p[:])
                    nc.sync.dma_start(z_shard[t * P:(t + 1) * P, :], zt[:])
                    if phases == "p1":
                        nc.vector.tensor_copy(
                            out_acc[:, t * C:(t + 1) * C], zt[:, :C]
                        )
                colbase += gcols

            def phase2():
                z_full = dpool.tile(
                    [ZT_ROWS, ZPAD], F32, name="z_full", addr_space="Shared"
                )
                # zero tail rows (phase-2 in-window zero rows)
                nc.sync.dma_start(z_shard[SROWS:SH_ROWS, :], zzero[0:ZTAIL, :])

                # ---------------- exchange z shards ----------------
                nc.gpsimd.collective_compute(
                    "AllGather",
                    mybir.AluOpType.bypass,
                    replica_groups=[list(range(NCORES))],
                    ins=[z_shard[:].opt()],
                    outs=[z_full[:].opt()],
                )

                # ---------------- phase 2 ----------------
                src2 = [z_full[BASES2[b]:, :] for b in range(NB)]
                colbase = 0
                for g, (t0, t1) in enumerate(G2):
                    GB, boffs, gcols, ntg = gather_group(
                        W2, g, t0, t1, i2_t, colbase, src2, ZPAD, f"2_{g}"
                    )
                    M2 = reduce_group(
                        W2, t0, t1, GB, boffs, ZPAD, ntg, f"2_{g}"
                    )
                    nmaxg = spool.tile([P, ntg], F32, name="nmaxg", tag="nmax")
                    ssumg = spool.tile([P, ntg], F32, name="ssumg", tag="ssum")
                    lseg = spool.tile([P, ntg], F32, name="lseg", tag="lse")
                    for t in range(t0, t1):
                        tl = t - t0
                        nc.vector.tensor_reduce(
                            out=nmaxg[:, tl:tl + 1],
                            in_=M2[:, tl, :C],
                            axis=AX.X,
                            op=ALU.max,
                            negate=True,
                        )
                        e = wpool.tile([P, C], F32, name="e", tag="e")
                        nc.scalar.activation(
                            e[:], M2[:, tl, :C], AF.Exp,
                            bias=nmaxg[:, tl:tl + 1],
                            accum_out=ssumg[:, tl:tl + 1],
                        )
                        nc.scalar.activation(
                            lseg[:, tl:tl + 1], ssumg[:, tl:tl + 1], AF.Ln
                        )
                    for t in range(t0, t1):
                        tl = t - t0
                        nc.vector.scalar_tensor_tensor(
                            out=out_acc[:, t * C:(t + 1) * C],
                            in0=M2[:, tl, :C],
                            scalar=nmaxg[:, tl:tl + 1],
                            in1=lseg[:, tl:tl + 1].to_broadcast([P, C]),
                            op0=ALU.add,
                            op1=ALU.subtract,
                        )
                    colbase += gcols

            for _rep in range(repeat):
                phase1()
                if phases == "full":
                    phase2()

            if phases in ("p1g", "p1gsep"):
                nc.vector.memset(out_acc[:], 0.0)
            nc.sync.dma_start(out_t.ap(), out_acc[:])

    nc.compile()
    return nc


# --------------------------------------------------------------------------
# kernel() entry
# --------------------------------------------------------------------------

_NC_CACHE = {}


def _prepare(neighbors, ids):
    """All host-side planning shared by program build and index build."""
    neighbors = np.asarray(neighbors).astype(np.int64)
    ids = np.asarray(ids).astype(np.int64)

    # phase 1: sources are original node ids in the zero-row-padded table
    pos1 = neighbors + neighbors // T1_BLOCK
    order1, W1, ebp1 = _plan(pos1, BASES1, ZROWS1)
    G1 = _groups(W1, CAP_COLS1, NTG1)

    # phase-1 rank -> z-table position
    rank1 = np.empty(N_NODES, np.int64)
    rank1[order1] = np.arange(N_NODES)
    zc_ = (rank1 // P) % NCORES
    zpos_of_node = zc_ * SH_ROWS + (rank1 // 1024) * P + rank1 % P

    # phase 2: output row i aggregates z of neighbors[ids[i]]
    nbr2 = neighbors[ids]
    pos2 = zpos_of_node[nbr2]
    order2, W2, ebp2 = _plan(pos2, BASES2, ZROWS2)
    G2 = _groups(W2, CAP_COLS2, NTG2)

    return {
        "order1": order1, "W1": W1, "ebp1": ebp1, "G1": G1,
        "order2": order2, "W2": W2, "ebp2": ebp2, "G2": G2,
    }


def make_host_inputs(feats, W0, b0, W1_, b1, ids, neighbors, plan=None):
    """Build per-core input maps (index prep + weight folding on host)."""
    if plan is None:
        plan = _prepare(neighbors, ids)

    feats = np.asarray(feats, np.float32)
    ftab = np.zeros((T1_ROWS, F), np.float32)
    for blk in range(4):
        s = blk * T1_BLOCK
        ftab[s + blk:s + blk + T1_BLOCK] = feats[s:s + T1_BLOCK]
    ftab = np.ascontiguousarray(ftab)

    w0s = np.ascontiguousarray(np.asarray(W0, np.float32) / DEG)
    w1s = np.ascontiguousarray(np.asarray(W1_, np.float32) / DEG)
    b0c = np.ascontiguousarray(np.asarray(b0, np.float32).reshape(H, 1))
    b1c = np.ascontiguousarray(np.asarray(b1, np.float32).reshape(C, 1) / DEG)

    in_maps = []
    for c in range(NCORES):
        i1 = _build_idx(plan["order1"], plan["ebp1"], plan["W1"],
                        plan["G1"], BASES1, c)
        i2 = _build_idx(plan["order2"], plan["ebp2"], plan["W2"],
                        plan["G2"], BASES2, c)
        in_maps.append({
            "feats": ftab, "w0": w0s, "b0": b0c, "w1": w1s, "b1": b1c,
            "i1": i1, "i2": i2,
        })
    return in_maps, plan


def io_signature(plan):
    """(name, shape, dtype, kind) of the program I/O -- for perf.build_null."""
    tot1 = int(plan["W1"].sum())
    tot2 = int(plan["W2"].sum())
    return [
        ("feats", [T1_ROWS, F], F32, "ExternalInput"),
        ("w0", [F, H], F32, "ExternalInput"),
        ("b0", [H, 1], F32, "ExternalInput"),
        ("w1", [H, C], F32, "ExternalInput"),
        ("b1", [C, 1], F32, "ExternalInput"),
        ("i1", [P, tot1 * 8], I16, "ExternalInput"),
        ("i2", [P, tot2 * 8], I16, "ExternalInput"),
        ("out", [P, TILES * C], F32, "ExternalOutput"),
    ]


def unshard_output(results, plan):
    """results: per-core {"out": [P, TILES*C]} -> full [N, C] in row order."""
    order2 = plan["order2"]
    out_full = np.empty((N_NODES, C), np.float32)
    for c in range(NCORES):
        o = np.asarray(results[c]["out"]).reshape(P, TILES, C)
        o = o.transpose(1, 0, 2).reshape(TILES * P, C)   # row t*128+p
        r = np.arange(TILES)[:, None] * 1024 + c * P + np.arange(P)[None, :]
        r = r.reshape(-1)
        valid = r < N_NODES
        out_full[order2[r[valid]]] = o[valid]
    return np.ascontiguousarray(out_full)


def _get_program(plan):
    key = (plan["W1"].tobytes(), plan["W2"].tobytes())
    if key not in _NC_CACHE:
        _NC_CACHE[key] = build_program(
            plan["W1"], plan["G1"], plan["W2"], plan["G2"], maxi=MAXI
        )
    return _NC_CACHE[key]


def kernel(**inputs):
    in_maps, plan = make_host_inputs(
        inputs["feats"], inputs["W0"], inputs["b0"], inputs["W1"],
        inputs["b1"], inputs["ids"], inputs["neighbors"],
    )
    nc = _get_program(plan)
    res = run_bass_kernel_spmd(nc, in_maps, core_ids=list(range(NCORES)))
    return unshard_output(res.results, plan)
